# revision 10
# baseline (speedup 1.0000x reference)
"""Equivariant interaction block (gnn message passing) on 8 trn2 NeuronCores.

v2.5 strategy (per-core, edges dst-sorted and sharded by dst node range):
  host prep: per-edge features pre-gathered and pre-scaled into xft
    (128 cols bf16: [xs*shs | C110*b | xv(d,i)*shs | xs], all scaled by
    1/deg[dst]); S selection matrices; radial basis transposed.
  pass 1 per 512-edge supertile:
    - radial MLP on PE (bias via ones-row), h2s [65, 512] bf16
    - per 128-edge sub-tile: W' = h2s_c @ w3p in 5 PSUM chunks; chunks
      0-2 copied to SBUF bf16 by Scalar, chunks 3-4 by Pool (gpsimd)
    - supertile-batched DVE: products + add-trees for the 4 CG paths,
      message assembly m_t [128, 4x80] bf16
    - S-matmul folds the within-tile segment-sum on PE; partials (bf16,
      inv-deg prescaled) written per supertile
  pass 2 per 512-node block: one indirect gather of 2 partial rows per
    node (zero-row for absent), one DVE add, write yN bf16.
"""

import os
import sys

import numpy as np

for _p in ("/opt/trn_rl_repo", os.path.expanduser("~/.axon_site/_ro/trn_rl_repo")):
    if os.path.isdir(_p) and _p not in sys.path:
        sys.path.insert(0, _p)

import concourse.bacc as bacc
import concourse.bass as bass
import concourse.mybir as mybir
import concourse.tile as tile
from concourse.bass_utils import run_bass_kernel_spmd

F32 = mybir.dt.float32
BF16 = mybir.dt.bfloat16
I32 = mybir.dt.int32
AF = mybir.ActivationFunctionType
OP = mybir.AluOpType

MUL0, MUL1 = 32, 16
RBF, HID = 16, 64
O1 = MUL0 * MUL0
O2 = O1 + MUL0 * MUL1
O3 = O2 + MUL1 * MUL1
WNUMEL = O3 + MUL1 * MUL0  # 2304
C_PATH = float(1.0 / np.sqrt(np.float32(MUL0 + MUL1)))
C_110 = float(1.0 / np.sqrt(3.0))
NCORES = 8
FDIM = MUL0 + 3 * MUL1  # 80
XC = 128  # xft cols: a=xs*shs 32 | C110*b 16 | xvs(d,i) 48 | xs 32
NSUB = 4  # 128-edge sub-tiles per supertile
SUPER = NSUB * 128


def _build_w3_perm():
    """Permutation + scale taking reference w3 columns into our layout.

    W' columns (2304):
      A [0,1536):    q = o*48 + j      (o in 32, j in 48)
          j < 32 : path1  W1[i=j, o]   -> src i*32+o          scale C_PATH
          j >= 32: path4  W4[i=j-32,o] -> src O3 + i*32+o     scale C_PATH*C110
      B [1536,2048): q = 1536 + o*32 + i (o in 16, i in 32)
          path2 W2[i, o] -> src O1 + i*16 + o                 scale C_PATH
      C [2048,2304): q = 2048 + o*16 + i (o in 16, i in 16)
          path3 W3[i, o] -> src O2 + i*16 + o                 scale C_PATH
    """
    src = np.zeros(WNUMEL, np.int64)
    scl = np.zeros(WNUMEL, np.float32)
    for o in range(MUL0):
        for j in range(48):
            q = o * 48 + j
            if j < 32:
                src[q] = j * MUL0 + o
                scl[q] = C_PATH
            else:
                src[q] = O3 + (j - 32) * MUL0 + o
                scl[q] = C_PATH * C_110
    for o in range(MUL1):
        for i in range(MUL0):
            q = 1536 + o * 32 + i
            src[q] = O1 + i * MUL1 + o
            scl[q] = C_PATH
    for o in range(MUL1):
        for i in range(MUL1):
            q = 2048 + o * 16 + i
            src[q] = O2 + i * MUL1 + o
            scl[q] = C_PATH
    return src, scl


def build_program(npc_pad, epad, num_cores):
    nsuper = epad // SUPER
    nblk = npc_pad // 128
    assert epad % SUPER == 0 and npc_pad % 128 == 0

    nc = bacc.Bacc(
        "TRN2",
        target_bir_lowering=False,
        debug=False,
        enable_asserts=False,
        num_devices=num_cores,
    )

    xft = nc.dram_tensor("xft", [nsuper * 128, NSUB * XC], BF16,
                         kind="ExternalInput")
    shvrep = nc.dram_tensor("shvrep", [nsuper * 128, NSUB * 48], BF16,
                            kind="ExternalInput")
    smat = nc.dram_tensor("smat", [nsuper * 128, NSUB * 128], BF16,
                          kind="ExternalInput")
    rbf17 = nc.dram_tensor("rbf17", [RBF + 1, epad], BF16, kind="ExternalInput")
    w1b = nc.dram_tensor("w1b", [RBF + 1, HID], BF16, kind="ExternalInput")
    w2b = nc.dram_tensor("w2b", [HID + 1, HID], BF16, kind="ExternalInput")
    w3p = nc.dram_tensor("w3p", [HID + 1, WNUMEL], BF16, kind="ExternalInput")
    g2 = nc.dram_tensor("g2", [nblk * 128, 2], I32, kind="ExternalInput")

    yN = nc.dram_tensor("yN", [npc_pad, FDIM], F32, kind="ExternalOutput")
    partials = nc.dram_tensor("partials", [epad + 128, FDIM], F32)

    with tile.TileContext(nc) as tc:
        with (
            nc.allow_low_precision(reason="bf16 per-edge messages and partials"),
            tc.tile_pool(name="const", bufs=1) as cp,
        ):
            w1b_sb = cp.tile([RBF + 1, HID], BF16)
            nc.sync.dma_start(out=w1b_sb[:], in_=w1b[:])
            w2b_sb = cp.tile([HID + 1, HID], BF16)
            nc.sync.dma_start(out=w2b_sb[:], in_=w2b[:])
            w3p_sb = cp.tile([HID + 1, WNUMEL], BF16)
            nc.sync.dma_start(out=w3p_sb[:], in_=w3p[:])
            zrow = cp.tile([128, FDIM], F32)
            nc.gpsimd.memset(zrow[:], 0.0)
            nc.sync.dma_start(out=partials[epad:epad + 128, :], in_=zrow[:])

            # ---------------- pass 1: edges ----------------
            # PSUM banks: wp 2x[128,1152]=6, mlp 1, cmb 1 -> 8 total
            with (
                tc.tile_pool(name="sb", bufs=2) as sp,
                tc.tile_pool(name="wps", bufs=2, space="PSUM") as wpp,
                tc.tile_pool(name="mlp", bufs=1, space="PSUM") as mpp,
                tc.tile_pool(name="cmb", bufs=1, space="PSUM") as cpp,
            ):
                for s in range(nsuper):
                    r0 = s * SUPER
                    rbf_t = sp.tile([RBF + 1, SUPER], BF16, tag="rbf")
                    nc.sync.dma_start(out=rbf_t[:], in_=rbf17[:, r0:r0 + SUPER])
                    xft_t = sp.tile([128, NSUB * XC], BF16, tag="xft")
                    nc.sync.dma_start(out=xft_t[:],
                                      in_=xft[s * 128:(s + 1) * 128, :])
                    shv_t = sp.tile([128, NSUB * 48], BF16, tag="shv")
                    nc.sync.dma_start(out=shv_t[:],
                                      in_=shvrep[s * 128:(s + 1) * 128, :])
                    s_t = sp.tile([128, NSUB * 128], BF16, tag="smat")
                    nc.sync.dma_start(out=s_t[:],
                                      in_=smat[s * 128:(s + 1) * 128, :])

                    # radial MLP
                    h1_ps = mpp.tile([HID, SUPER], F32, tag="mlp")
                    nc.tensor.matmul(out=h1_ps[:], lhsT=w1b_sb[:], rhs=rbf_t[:],
                                     start=True, stop=True)
                    h1s = sp.tile([HID + 1, SUPER], BF16, tag="h1s")
                    nc.scalar.activation(h1s[:HID, :], h1_ps[:], AF.Silu)
                    nc.gpsimd.memset(h1s[HID:HID + 1, :], 1.0)
                    h2_ps = mpp.tile([HID, SUPER], F32, tag="mlp")
                    nc.tensor.matmul(out=h2_ps[:], lhsT=w2b_sb[:], rhs=h1s[:],
                                     start=True, stop=True)
                    h2s = sp.tile([HID + 1, SUPER], BF16, tag="h2s")
                    nc.scalar.activation(h2s[:HID, :], h2_ps[:], AF.Silu)
                    nc.gpsimd.memset(h2s[HID:HID + 1, :], 1.0)

                    # W' per sub-tile: 2 fat PSUM chunks of 1152 cols, each
                    # filled by 3 bank-aligned matmuls, then one fat Scalar
                    # copy (f32 -> bf16) per chunk.
                    wsb = sp.tile([128, NSUB * WNUMEL], BF16, tag="wsb")
                    for c in range(NSUB):
                        lhs = h2s[:, c * 128:(c + 1) * 128]
                        for half in range(2):
                            q0 = half * 1152
                            w_ps = wpp.tile([128, 1536], F32, tag="w")
                            for k0, k1 in ((0, 512), (512, 1024), (1024, 1152)):
                                nc.tensor.matmul(
                                    out=w_ps[:, k0:k1], lhsT=lhs,
                                    rhs=w3p_sb[:, q0 + k0:q0 + k1],
                                    start=True, stop=True)
                            nc.scalar.copy(
                                wsb[:, c * WNUMEL + q0:c * WNUMEL + q0 + 1152],
                                w_ps[:, 0:1152])

                    # ---- supertile-batched DVE ----
                    xf4 = xft_t[:].rearrange("p (s f) -> p s f", f=XC)
                    wsb4 = wsb[:].rearrange("p (s q) -> p s q", q=WNUMEL)

                    m_t = sp.tile([128, NSUB * FDIM], BF16, tag="m")
                    m4 = m_t[:].rearrange("p (s f) -> p s f", f=FDIM)

                    # products
                    pa = sp.tile([128, NSUB * 1536], BF16, tag="pa")
                    nc.vector.tensor_tensor(
                        out=pa[:].rearrange("p (s o j) -> p s o j", o=32, j=48),
                        in0=wsb4[:, :, 0:1536].rearrange(
                            "p s (o j) -> p s o j", j=48),
                        in1=xf4[:, :, 0:48].unsqueeze(2)
                            .to_broadcast([128, NSUB, 32, 48]),
                        op=OP.mult)
                    pb = sp.tile([128, NSUB * 512], BF16, tag="pb")
                    nc.vector.tensor_tensor(
                        out=pb[:].rearrange("p (s o i) -> p s o i", o=16, i=32),
                        in0=wsb4[:, :, 1536:2048].rearrange(
                            "p s (o i) -> p s o i", i=32),
                        in1=xf4[:, :, 96:128].unsqueeze(2)
                            .to_broadcast([128, NSUB, 16, 32]),
                        op=OP.mult)
                    pc = sp.tile([128, NSUB * 768], BF16, tag="pc")
                    for c in range(NSUB):
                        nc.vector.tensor_tensor(
                            out=pc[:, c * 768:(c + 1) * 768].rearrange(
                                "p (d o i) -> p d o i", d=3, i=16),
                            in0=wsb[:, c * WNUMEL + 2048:c * WNUMEL + 2304]
                                .rearrange("p (o i) -> p o i", i=16)
                                .unsqueeze(1).to_broadcast([128, 3, 16, 16]),
                            in1=xft_t[:, c * XC + 48:c * XC + 96]
                                .rearrange("p (d i) -> p d i", i=16)
                                .unsqueeze(2).to_broadcast([128, 3, 16, 16]),
                            op=OP.mult)

                    # A tree: 48 -> 24 -> 12 -> 6 -> 3 -> reduce3
                    pa4 = pa[:].rearrange("p (s o j) -> p s o j", o=32, j=48)
                    ta1 = sp.tile([128, NSUB * 768], BF16, tag="ta1")
                    t1v = ta1[:].rearrange("p (s o j) -> p s o j", o=32, j=24)
                    nc.vector.tensor_tensor(out=t1v, in0=pa4[:, :, :, 0:24],
                                            in1=pa4[:, :, :, 24:48], op=OP.add)
                    ta2 = sp.tile([128, NSUB * 384], BF16, tag="ta2")
                    t2v = ta2[:].rearrange("p (s o j) -> p s o j", o=32, j=12)
                    nc.vector.tensor_tensor(out=t2v, in0=t1v[:, :, :, 0:12],
                                            in1=t1v[:, :, :, 12:24], op=OP.add)
                    ta3 = sp.tile([128, NSUB * 192], BF16, tag="ta3")
                    t3v = ta3[:].rearrange("p (s o j) -> p s o j", o=32, j=6)
                    nc.vector.tensor_tensor(out=t3v, in0=t2v[:, :, :, 0:6],
                                            in1=t2v[:, :, :, 6:12], op=OP.add)
                    ta4 = sp.tile([128, NSUB * 96], BF16, tag="ta4")
                    t4v = ta4[:].rearrange("p (s o j) -> p s o j", o=32, j=3)
                    nc.vector.tensor_tensor(out=t4v, in0=t3v[:, :, :, 0:3],
                                            in1=t3v[:, :, :, 3:6], op=OP.add)
                    nc.vector.tensor_reduce(
                        out=m4[:, :, 0:32], in_=t4v,
                        axis=mybir.AxisListType.X, op=OP.add)

                    # B tree: 32 -> 16 -> 8 -> 4 -> 2 -> add
                    pb4 = pb[:].rearrange("p (s o i) -> p s o i", o=16, i=32)
                    tb1 = sp.tile([128, NSUB * 256], BF16, tag="tb1")
                    b1v = tb1[:].rearrange("p (s o i) -> p s o i", o=16, i=16)
                    nc.vector.tensor_tensor(out=b1v, in0=pb4[:, :, :, 0:16],
                                            in1=pb4[:, :, :, 16:32], op=OP.add)
                    tb2 = sp.tile([128, NSUB * 128], BF16, tag="tb2")
                    b2v = tb2[:].rearrange("p (s o i) -> p s o i", o=16, i=8)
                    nc.vector.tensor_tensor(out=b2v, in0=b1v[:, :, :, 0:8],
                                            in1=b1v[:, :, :, 8:16], op=OP.add)
                    tb3 = sp.tile([128, NSUB * 64], BF16, tag="tb3")
                    b3v = tb3[:].rearrange("p (s o i) -> p s o i", o=16, i=4)
                    nc.vector.tensor_tensor(out=b3v, in0=b2v[:, :, :, 0:4],
                                            in1=b2v[:, :, :, 4:8], op=OP.add)
                    tb4 = sp.tile([128, NSUB * 32], BF16, tag="tb4")
                    b4v = tb4[:].rearrange("p (s o i) -> p s o i", o=16, i=2)
                    nc.vector.tensor_tensor(out=b4v, in0=b3v[:, :, :, 0:2],
                                            in1=b3v[:, :, :, 2:4], op=OP.add)
                    t2s = sp.tile([128, NSUB * 16], BF16, tag="t2s")
                    t2s4 = t2s[:].rearrange("p (s o) -> p s o", o=16)
                    nc.vector.tensor_tensor(out=t2s4, in0=b4v[:, :, :, 0],
                                            in1=b4v[:, :, :, 1], op=OP.add)

                    # C tree: 16 -> 8 -> 4 -> 2 -> add  (groups (s,d,o))
                    pc4 = pc[:].rearrange("p (s g i) -> p s g i", g=48, i=16)
                    tc1 = sp.tile([128, NSUB * 384], BF16, tag="tc1")
                    c1v = tc1[:].rearrange("p (s g i) -> p s g i", g=48, i=8)
                    nc.vector.tensor_tensor(out=c1v, in0=pc4[:, :, :, 0:8],
                                            in1=pc4[:, :, :, 8:16], op=OP.add)
                    tc2 = sp.tile([128, NSUB * 192], BF16, tag="tc2")
                    c2v = tc2[:].rearrange("p (s g i) -> p s g i", g=48, i=4)
                    nc.vector.tensor_tensor(out=c2v, in0=c1v[:, :, :, 0:4],
                                            in1=c1v[:, :, :, 4:8], op=OP.add)
                    tc3 = sp.tile([128, NSUB * 96], BF16, tag="tc3")
                    c3v = tc3[:].rearrange("p (s g i) -> p s g i", g=48, i=2)
                    nc.vector.tensor_tensor(out=c3v, in0=c2v[:, :, :, 0:2],
                                            in1=c2v[:, :, :, 2:4], op=OP.add)
                    v3 = sp.tile([128, NSUB * 48], BF16, tag="v3")
                    v34 = v3[:].rearrange("p (s g) -> p s g", g=48)
                    nc.vector.tensor_tensor(out=v34, in0=c3v[:, :, :, 0],
                                            in1=c3v[:, :, :, 1], op=OP.add)

                    # m_v = t2 (x) sh_v  +  v3
                    mtmp = sp.tile([128, NSUB * 48], BF16, tag="mtmp")
                    nc.gpsimd.tensor_tensor(
                        out=mtmp[:].rearrange("p (s d o) -> p s d o", d=3, o=16),
                        in0=t2s4.unsqueeze(2).to_broadcast([128, NSUB, 3, 16]),
                        in1=shv_t[:].rearrange("p (s d o) -> p s d o",
                                               d=3, o=16),
                        op=OP.mult)
                    nc.vector.tensor_tensor(
                        out=m4[:, :, 32:80],
                        in0=mtmp[:].rearrange("p (s g) -> p s g", g=48),
                        in1=v34, op=OP.add)

                    # S-combine on PE, per sub-tile
                    comb_ps = cpp.tile([128, 512], F32, tag="comb")
                    for c in range(NSUB):
                        nc.tensor.matmul(
                            out=comb_ps[:, c * FDIM:(c + 1) * FDIM],
                            lhsT=s_t[:, c * 128:(c + 1) * 128],
                            rhs=m_t[:, c * FDIM:(c + 1) * FDIM],
                            start=True, stop=True)
                    comb_sb = sp.tile([128, NSUB * FDIM], F32, tag="combsb")
                    nc.scalar.copy(comb_sb[:], comb_ps[:, 0:NSUB * FDIM])
                    nc.sync.dma_start(
                        out=partials[r0:r0 + SUPER, :].rearrange(
                            "(c p) f -> p c f", c=NSUB),
                        in_=comb_sb[:].rearrange("p (c f) -> p c f", c=NSUB))

            # ---------------- pass 2: nodes (128 per block) ----------------
            with (
                tc.tile_pool(name="sb2", bufs=8) as s2,
            ):
                for b in range(nblk):
                    n0 = b * 128
                    g_t = s2.tile([128, 2], I32, tag="g")
                    nc.sync.dma_start(out=g_t[:], in_=g2[n0:n0 + 128, :])
                    p1 = s2.tile([128, FDIM], F32, tag="p1")
                    nc.gpsimd.indirect_dma_start(
                        out=p1[:], out_offset=None, in_=partials[:],
                        in_offset=bass.IndirectOffsetOnAxis(
                            ap=g_t[:, 0:1], axis=0))
                    p2 = s2.tile([128, FDIM], F32, tag="p2")
                    nc.gpsimd.indirect_dma_start(
                        out=p2[:], out_offset=None, in_=partials[:],
                        in_offset=bass.IndirectOffsetOnAxis(
                            ap=g_t[:, 1:2], axis=0))
                    yb = s2.tile([128, FDIM], F32, tag="yb")
                    nc.vector.tensor_tensor(out=yb[:], in0=p1[:], in1=p2[:],
                                            op=OP.add)
                    nc.sync.dma_start(out=yN[n0:n0 + 128, :], in_=yb[:])

    nc.compile()
    return nc


_PROGRAM_CACHE = {}


def _get_program(npc_pad, epad, num_cores):
    key = (npc_pad, epad, num_cores)
    if key not in _PROGRAM_CACHE:
        _PROGRAM_CACHE[key] = build_program(npc_pad, epad, num_cores)
    return _PROGRAM_CACHE[key]


def prepare_in_maps(x, edge_src, edge_dst, edge_sh, edge_rbf,
                    w1, b1, w2, b2, w3, b3, ws_out, wv_out, num_cores=NCORES):
    n = x.shape[0]
    npc = -(-n // num_cores)
    npc_pad = -(-npc // 512) * 512

    dst = np.asarray(edge_dst, np.int64)
    src = np.asarray(edge_src, np.int64)
    order = np.argsort(dst, kind="stable")
    dst_s = dst[order]
    src_s = src[order]
    sh_s = np.asarray(edge_sh, np.float32)[order]
    rbf_s = np.asarray(edge_rbf, np.float32)[order]

    bounds = np.searchsorted(dst_s, np.arange(num_cores + 1) * npc)
    counts = np.diff(bounds)
    epad = max(SUPER, int(-(-counts.max() // SUPER) * SUPER))
    nsuper = epad // SUPER

    bf16 = mybir.dt.np(BF16)
    w1bh = np.concatenate([np.asarray(w1, np.float32),
                           np.asarray(b1, np.float32)[None, :]], 0).astype(bf16)
    w2bh = np.concatenate([np.asarray(w2, np.float32),
                           np.asarray(b2, np.float32)[None, :]], 0).astype(bf16)
    perm, scl = _build_w3_perm()
    w3p_f = np.concatenate(
        [np.asarray(w3, np.float32)[:, perm] * scl[None, :],
         (np.asarray(b3, np.float32)[perm] * scl)[None, :]], 0)
    # fold the output irrep-linear into the per-edge TP weights
    wso = np.asarray(ws_out, np.float32)
    wvo = np.asarray(wv_out, np.float32)
    A = w3p_f[:, 0:1536].reshape(HID + 1, 32, 48)
    w3p_f[:, 0:1536] = np.einsum("hoj,oq->hqj", A, wso).reshape(HID + 1, 1536)
    B = w3p_f[:, 1536:2048].reshape(HID + 1, 16, 32)
    w3p_f[:, 1536:2048] = np.einsum("hoi,oq->hqi", B, wvo).reshape(HID + 1, 512)
    C = w3p_f[:, 2048:2304].reshape(HID + 1, 16, 16)
    w3p_f[:, 2048:2304] = np.einsum("hoi,oq->hqi", C, wvo).reshape(HID + 1, 256)
    w3ph = w3p_f.astype(bf16)
    xf = np.asarray(x, np.float32)
    xv_di = xf[:, MUL0:].reshape(n, MUL1, 3).transpose(0, 2, 1).reshape(n, 48)

    # per-node degree (global, on sorted dst)
    deg_all = np.zeros(n + 1, np.float32)
    np.add.at(deg_all, dst_s, 1.0)
    inv_all = 1.0 / np.maximum(deg_all, 1.0)

    in_maps = []
    meta = {"npc": npc, "npc_pad": npc_pad, "epad": epad, "n": n,
            "num_cores": num_cores}
    for c in range(num_cores):
        lo, hi = bounds[c], bounds[c + 1]
        ec = hi - lo
        csrc = src_s[lo:hi]
        cdst = np.full(epad, -1, np.int64)
        cdst[:ec] = dst_s[lo:hi]
        csh = np.zeros((epad, 4), np.float32)
        csh[:ec] = sh_s[lo:hi]
        cinv = np.zeros((epad, 1), np.float32)
        cinv[:ec, 0] = inv_all[dst_s[lo:hi]]

        crbf = np.zeros((RBF + 1, epad), np.float32)
        crbf[:RBF, :ec] = rbf_s[lo:hi].T
        crbf[RBF, :] = 1.0
        crbf = crbf.astype(bf16)

        # per-edge features [epad, 128]:
        # [a=xs*shs 32 | C110*b 16 | xvs=xv(d,i)*shs 48 | xs 32], * invdeg
        feat = np.zeros((epad, XC), np.float32)
        xs_e = xf[csrc, :MUL0]                      # [ec, 32]
        xvdi_e = xv_di[csrc]                        # [ec, 48] (d,i)
        xvid_e = xf[csrc, MUL0:]                    # [ec, 48] (i,d)
        shs_e = csh[:ec, 0:1]
        shv_e = csh[:ec, 1:4]                       # [ec, 3]
        feat[:ec, 0:32] = xs_e * shs_e
        b_e = np.einsum("eid,ed->ei",
                        xvid_e.reshape(ec, MUL1, 3), shv_e)
        feat[:ec, 32:48] = C_110 * b_e
        feat[:ec, 48:96] = xvdi_e * shs_e
        feat[:ec, 96:128] = xs_e
        feat[:ec] *= cinv[:ec]

        # shv repeated in (d, o) layout (not inv-scaled)
        svr = np.repeat(csh[:, 1:4], 16, axis=1)    # [epad, 48]

        # S matrices per 128-edge tile
        d2 = cdst.reshape(-1, 128)
        S = (d2[:, :, None] == d2[:, None, :]) & (d2[:, :, None] >= 0)
        S = S.astype(np.float32)

        # reorder edge-major [epad] -> [nsuper*128, NSUB*width]
        def to_g(a, width):
            a = a.reshape(nsuper, NSUB, 128, width)
            return a.transpose(0, 2, 1, 3).reshape(nsuper * 128, NSUB * width)

        feat_g = to_g(feat, XC).astype(bf16)
        svr_g = to_g(svr, 48).astype(bf16)
        S_g = to_g(S.reshape(epad, 128), 128).astype(bf16)

        # pass-2 gather rows: first/last partial row per node (zero row if
        # absent / degenerate). zero row = epad (explicitly zeroed on device).
        nbase = c * npc
        nodes = np.arange(npc_pad, dtype=np.int64) + nbase
        first = np.searchsorted(dst_s[lo:hi], nodes, side="left")
        last = np.searchsorted(dst_s[lo:hi], nodes, side="right") - 1
        has = last >= first
        spans = has & ((first // 128) != (last // 128))
        g = np.full((npc_pad, 2), epad, np.int32)
        g[has, 0] = first[has].astype(np.int32)
        g[spans, 1] = last[spans].astype(np.int32)
        gg = g.astype(np.int32)

        in_maps.append({
            "xft": feat_g, "shvrep": svr_g, "smat": S_g, "rbf17": crbf,
            "w1b": w1bh, "w2b": w2bh, "w3p": w3ph, "g2": gg,
        })
    return in_maps, meta


def kernel(x, edge_src, edge_dst, edge_sh, edge_rbf,
           w1, b1, w2, b2, w3, b3, ws_self, wv_self, ws_out, wv_out,
           _trace=False):
    num_cores = NCORES
    in_maps, meta = prepare_in_maps(
        x, edge_src, edge_dst, edge_sh, edge_rbf, w1, b1, w2, b2, w3, b3,
        ws_out, wv_out, num_cores=num_cores)

    nc = _get_program(meta["npc_pad"], meta["epad"], num_cores)
    res = run_bass_kernel_spmd(nc, in_maps, list(range(num_cores)),
                               trace=_trace)

    # self path in f32 on host (exact)
    xf = np.asarray(x, np.float32)
    n, npc = meta["n"], meta["npc"]
    ys = xf[:, :MUL0] @ np.asarray(ws_self, np.float32)
    yv = np.einsum("nid,io->nod", xf[:, MUL0:].reshape(n, MUL1, 3),
                   np.asarray(wv_self, np.float32))
    y = np.concatenate([ys, yv.reshape(n, 3 * MUL1)], axis=1)
    for c in range(num_cores):
        lo = c * npc
        hi = min(lo + npc, n)
        aggc = np.asarray(res.results[c]["yN"]).astype(np.float32)[:hi - lo]
        # agg layout L: [s(32) | v(d,o)] -> reference cols [s | v(o,d)]
        y[lo:hi, :MUL0] += aggc[:, :MUL0]
        v = aggc[:, MUL0:].reshape(hi - lo, 3, MUL1)
        y[lo:hi, MUL0:] += v.transpose(0, 2, 1).reshape(hi - lo, 48)
    kernel._last_results = res
    return y
